# revision 11
# baseline (speedup 1.0000x reference)
"""Two-layer GCN block (PyG GCNConv x2) on 8 trn2 NeuronCores.

Math: out1 = D^-1/2 (A+I) D^-1/2 (x W1) + b1 ; out2 = same on out1 with W2, b2.
Factorization on device (dest-sharded, 6250 dests/core, degree-sorted tiles):
    u    = dis (.) x                        (host; dis = deg^-1/2)
    A[d] = sum_{e: s->d} u[s] + dis_d x_d   (batched dma_gather + PSUM matmuls)
    v    = dis^2 (.) A @ W1 + dis (.) b1    (= dis (.) out1, gathered by layer 2)
    out2 = dis (.) (A2 @ W2) + b2
Gathers use the SWDGE dma_gather instruction (one instruction per
(group, window) instead of one indirect DMA per slot).  int16 gather
indices limit each gather to a 32768-row window, so each layer's table
exists in two copies (layer 1: host-built rotation; layer 2: two
AllGather layouts with different piece boundaries) and six windows,
with per-group window slot counts from a small LP.
"""
import sys
import numpy as np

sys.path.insert(0, '/root/.axon_site')
sys.path.insert(0, '/opt/trn_rl_repo')

N = 50000
E = 800000
D = 64
C = 8
NSH = 6250
P = 128
NT = 49                 # dest tiles per core
NSHP = NT * P           # 6272
WSZ = 32768
GROUPS = [(g * 4, 4) for g in range(12)] + [(48, 1)]
NW = 6                  # gather windows per layer

# layer-1 table: nodes + 5 interspersed zero rows, plus a rotated copy
NZ1 = 5
NTAB1 = N + NZ1         # 50005
ZP1 = np.array([0, 12501, 25002, 37503, 50004], dtype=np.int64)
ROT = 25003
B1 = NTAB1 - WSZ        # 17237

# layer-2 tables: two AllGather layouts (piece-major, rank-major regions
# with a zero row before each region and one at the end)
TB_A = [0, 20, 36, 48, 49]
TB_B = [0, 12, 24, 36, 48, 49]
# fire copy-piece after these group indices (group g covers tiles [4g,4g+4))
FIRE = {2: [(1, 0)], 4: [(0, 0)], 5: [(1, 1)], 8: [(0, 1), (1, 2)],
        11: [(0, 2), (1, 3)], 12: [(0, 3), (1, 4)]}

_compiled = None


def _vlayout(TB):
    """zero positions, region bases, per-core piece rows for a v copy."""
    pr = [(TB[i + 1] - TB[i]) * P for i in range(len(TB) - 1)]
    zpos, base = [], []
    cur = 0
    for i in range(len(pr)):
        zpos.append(cur)
        cur += 1
        base.append(cur)
        cur += C * pr[i]
    zpos.append(cur)
    cur += 1
    return pr, np.array(zpos), np.array(base), cur  # cur = NTAB


def _win_zero(zpos_sorted, b):
    """a zero-row local index inside window [b, b+WSZ)"""
    for z in zpos_sorted:
        if b <= z < b + WSZ:
            return int(z - b)
    raise AssertionError(f"no zero row in window base {b}")


def _lp_slots(maxcnt, nw):
    """min sum D_w s.t. sum_{w in S} D_w >= maxcnt[S] for all subsets S."""
    from scipy.optimize import linprog
    Aub, bub = [], []
    for S in range(1, 1 << nw):
        if maxcnt[S] == 0:
            continue
        Aub.append([-(1.0 if (S >> w) & 1 else 0.0) for w in range(nw)])
        bub.append(-float(maxcnt[S]))
    if not Aub:
        return np.zeros(nw, dtype=np.int64)
    res = linprog(c=np.ones(nw), A_ub=Aub, b_ub=bub,
                  bounds=[(0, None)] * nw, method='highs')
    Ds = np.ceil(res.x - 1e-9).astype(np.int64)
    for _ in range(30):
        ok = True
        for S in range(1, 1 << nw):
            need = maxcnt[S] - sum(Ds[w] for w in range(nw) if (S >> w) & 1)
            if need > 0:
                wmax = max((w for w in range(nw) if (S >> w) & 1),
                           key=lambda w: Ds[w])
                Ds[wmax] += need
                ok = False
        if ok:
            break
    return Ds


_POP = np.array([bin(m).count('1') for m in range(1 << NW)])


def _schedule_layer(posW, bases, rs_by_core, starts_by_core, perms):
    """Per-group window slot counts + per (group, core) source->slot values.

    posW: [NW, N] position of each node in each window's copy coordinates
          (already base-subtracted; -1 if outside the window)
    Returns groups_meta: list of dicts with D (per-window slots) and
    vals[k][w] = int16 [D_w, ntile, P] arrays (index values, pad = zero row).
    """
    inwin = posW >= 0                      # [NW, N]
    masks_all = np.zeros(N, dtype=np.int64)
    for w in range(NW):
        masks_all |= inwin[w].astype(np.int64) << w
    assert (masks_all > 0).all()

    out = []
    for gi, (t0, ntile) in enumerate(GROUPS):
        lo, hi = t0 * P, min((t0 + ntile) * P, NSH)
        # subset maxima over all cores' dests
        cnt = np.zeros((C * (hi - lo), 1 << NW), dtype=np.int64)
        src_lists = []   # per core: (dsts, per-dest source arrays)
        for k in range(C):
            dsts = perms[k][lo:hi]
            st = starts_by_core[k]
            rs = rs_by_core[k]
            row_i = []
            for di, dv in enumerate(dsts):
                srcs = rs[st[dv]:st[dv + 1]]
                row_i.append(srcs)
                if len(srcs):
                    m, c_ = np.unique(masks_all[srcs], return_counts=True)
                    cnt[k * (hi - lo) + di, m] = c_
            src_lists.append((dsts, row_i))
        # zeta transform: cnt[:, S] = #sources with mask subset of S
        for w in range(NW):
            bit = 1 << w
            idx = np.arange(1 << NW)
            sel = (idx & bit) != 0
            cnt[:, idx[sel]] += cnt[:, idx[sel] ^ bit]
        maxcnt = cnt.max(axis=0)
        Ds = _lp_slots(maxcnt, NW)

        # greedy per-dest assignment with caps Ds (bump on failure)
        for _attempt in range(50):
            vals = [[np.full((int(Ds[w]), ntile, P), -1, dtype=np.int64)
                     for w in range(NW)] for _ in range(C)]
            failed = None
            for k in range(C):
                dsts, row_i = src_lists[k]
                for di, srcs in enumerate(row_i):
                    if len(srcs) == 0:
                        continue
                    q, p = di // P, di % P
                    mks = masks_all[srcs]
                    order = np.argsort(_POP[mks], kind='stable')
                    used = np.zeros(NW, dtype=np.int64)
                    for si in order:
                        m = mks[si]
                        best, bestrem = -1, 0
                        for w in range(NW):
                            if (m >> w) & 1:
                                rem = Ds[w] - used[w]
                                if rem > bestrem:
                                    best, bestrem = w, rem
                        if best < 0:
                            failed = m
                            break
                        vals[k][best][used[best], q, p] = posW[best][srcs[si]]
                        used[best] += 1
                    if failed is not None:
                        break
                if failed is not None:
                    break
            if failed is None:
                break
            w_b = max((w for w in range(NW) if (failed >> w) & 1),
                      key=lambda w: Ds[w])
            Ds[w_b] += 1
        assert failed is None, "assignment failed after bumps"
        out.append({"t0": t0, "ntile": ntile, "D": Ds, "vals": vals})
    return out


def _build_idx_host(sched, zlocs):
    """Pack per-(group, window) values into the int16 SBUF index layout."""
    cols_total = sum(8 * int(m["D"][w]) * m["ntile"]
                     for m in sched for w in range(NW))
    idx = np.zeros((C, P, cols_total), dtype=np.int16)
    offs = []     # per group: list of (w, col_off, ncols, num_idxs)
    c0 = 0
    for m in sched:
        ntile = m["ntile"]
        go = []
        for w in range(NW):
            Dw = int(m["D"][w])
            if Dw == 0:
                continue
            ncols = 8 * Dw * ntile
            n_idx = P * Dw * ntile
            for k in range(C):
                v = m["vals"][k][w]          # [Dw, ntile, P]
                v = np.where(v < 0, zlocs[w], v)
                flat = v.reshape(-1).astype(np.int16)      # (s, q, p) order
                blk = flat.reshape(-1, 16).T               # [16, ncols]
                idx[k, :, c0:c0 + ncols] = np.tile(blk, (8, 1))
            go.append((w, c0, ncols, n_idx))
            c0 += ncols
        offs.append(go)
    return idx, offs, cols_total


def kernel(x, edge_index, W1, b1, W2, b2):
    import concourse.bass as bass
    import concourse.bacc as bacc
    import concourse.mybir as mybir
    from concourse import tile
    from concourse.library_config import mlp
    from concourse.bass_utils import run_bass_kernel_spmd

    x = np.asarray(x, dtype=np.float32)
    edge_index = np.asarray(edge_index)
    W1 = np.asarray(W1, dtype=np.float32)
    W2 = np.asarray(W2, dtype=np.float32)
    b1 = np.asarray(b1, dtype=np.float32).reshape(1, D)
    b2 = np.asarray(b2, dtype=np.float32).reshape(1, D)

    row = edge_index[0].astype(np.int64)
    col = edge_index[1].astype(np.int64)
    deg = np.bincount(col, minlength=N).astype(np.float32) + 1.0
    dis = (1.0 / np.sqrt(deg)).astype(np.float32)

    # ---- per-core edge lists (dest-sharded), degree-sorted dest tiles ----
    order = np.argsort(col, kind='stable')
    col_s, row_s = col[order], row[order]
    bounds = np.searchsorted(col_s, np.arange(0, N + 1, NSH))
    perms, pinvs, starts_by_core, rs_by_core = [], [], [], []
    for k in range(C):
        sl = slice(bounds[k], bounds[k + 1])
        lc = col_s[sl] - k * NSH
        dd = np.bincount(lc, minlength=NSH)
        perm = np.argsort(-dd, kind='stable')
        pinv = np.empty(NSH, dtype=np.int64)
        pinv[perm] = np.arange(NSH)
        perms.append(perm)
        pinvs.append(pinv)
        starts_by_core.append(np.searchsorted(lc, np.arange(NSH + 1)))
        rs_by_core.append(row_s[sl])

    # ---- layer-1 tables (u = dis * x, two copies, interspersed zeros) ----
    pos1 = np.delete(np.arange(NTAB1), ZP1)          # node i -> T1 row
    pos1r = (pos1 - ROT) % NTAB1                     # node i -> T1R row
    u = dis[:, None] * x
    T1 = np.zeros((NTAB1, D), dtype=np.float32)
    T1[pos1] = u
    T1R = np.zeros((NTAB1, D), dtype=np.float32)
    T1R[pos1r] = u
    zp1r_sorted = np.sort((ZP1 - ROT) % NTAB1)
    bases1 = [0, B1 // 2, B1, 0, B1 // 2, B1]        # windows 0-2: T1, 3-5: T1R
    zlocs1 = [_win_zero(ZP1, bases1[w]) if w < 3 else
              _win_zero(zp1r_sorted, bases1[w]) for w in range(NW)]
    posW1 = np.full((NW, N), -1, dtype=np.int64)
    for w in range(NW):
        p_ = (pos1 if w < 3 else pos1r) - bases1[w]
        posW1[w] = np.where((p_ >= 0) & (p_ < WSZ), p_, -1)

    # ---- layer-2 tables: two v layouts ----
    prA, zposA, baseA, NTA = _vlayout(TB_A)
    prB, zposB, baseB, NTB = _vlayout(TB_B)
    B2a, B2b = NTA - WSZ, NTB - WSZ

    def vpos(TB, pr, base_arr):
        # node -> position in this copy
        pos = np.empty(N, dtype=np.int64)
        tbp = np.array(TB[1:]) * P
        for k in range(C):
            p = pinvs[k][np.arange(NSH)]
            pc = np.searchsorted(tbp, p, side='right')
            pos[k * NSH:(k + 1) * NSH] = (base_arr[pc] + k * np.array(pr)[pc]
                                          + (p - np.array(TB)[pc] * P))
        return pos

    pos2a = vpos(TB_A, prA, baseA)
    pos2b = vpos(TB_B, prB, baseB)
    bases2 = [0, B2a // 2, B2a, 0, B2b // 2, B2b]
    zlocs2 = [_win_zero(zposA, bases2[w]) if w < 3 else
              _win_zero(zposB, bases2[w]) for w in range(NW)]
    posW2 = np.full((NW, N), -1, dtype=np.int64)
    for w in range(NW):
        p_ = (pos2a if w < 3 else pos2b) - bases2[w]
        posW2[w] = np.where((p_ >= 0) & (p_ < WSZ), p_, -1)

    # ---- schedules + index arrays ----
    sched1 = _schedule_layer(posW1, bases1, rs_by_core, starts_by_core, perms)
    sched2 = _schedule_layer(posW2, bases2, rs_by_core, starts_by_core, perms)
    idx1_host, offs1, IW1 = _build_idx_host(sched1, zlocs1)
    idx2_host, offs2, IW2 = _build_idx_host(sched2, zlocs2)
    maxblk = max(int(m["D"].sum()) * m["ntile"] for m in sched1 + sched2)

    # ---- per-core dense arrays ----
    NPAIR = (NT + 1) // 2                            # 25 pair slots
    self1_arr = np.zeros((C, NSHP, D), dtype=np.float32)
    disq1_arr = np.zeros((C, P, NT * D), dtype=np.float32)
    disq2_arr = np.zeros((C, P, NT * D), dtype=np.float32)
    disrow_arr = np.zeros((C, 2, NPAIR * P), dtype=np.float32)
    for k in range(C):
        dval = dis[k * NSH + perms[k]]               # dis by sorted position
        self1_arr[k, :NSH] = dval[:, None] * x[k * NSH + perms[k]]
        dpad = np.zeros(NSHP, dtype=np.float32)
        dpad[:NSH] = dval
        dt_ = dpad.reshape(NT, P)                    # [tile, dest-in-tile]
        disq1_arr[k] = np.repeat(dt_.T ** 2, D, axis=1)
        disq2_arr[k] = np.repeat(dt_.T, D, axis=1)
        disrow_arr[k, 0, :] = dt_[0::2].reshape(-1)
        disrow_arr[k, 1, :NT // 2 * P] = dt_[1::2].reshape(-1)

    wblk1 = np.zeros((P, P), dtype=np.float32)
    wblk1[:D, :D] = W1
    wblk1[D:, D:] = W1
    wblk2 = np.zeros((P, P), dtype=np.float32)
    wblk2[:D, :D] = W2
    wblk2[D:, D:] = W2
    bp1 = np.zeros((2, P), dtype=np.float32)
    bp1[0, :D] = b1[0]
    bp1[1, D:] = b1[0]
    bp2 = np.zeros((2, P), dtype=np.float32)
    bp2[0, :D] = b2[0]
    bp2[1, D:] = b2[0]

    # ---- device program ----
    nc = bacc.Bacc(None, target_bir_lowering=False)
    dt = mybir.dt
    f32, f32r, i16 = dt.float32, dt.float32r, dt.int16
    t1p = nc.declare_dram_parameter("t1p", [NTAB1, D], f32r, isOutput=False)
    t1rp = nc.declare_dram_parameter("t1rp", [NTAB1, D], f32r, isOutput=False)
    self1p = nc.declare_dram_parameter("self1p", [NT, P, D], f32r, isOutput=False)
    idx1p = nc.declare_dram_parameter("idx1p", [P, IW1], i16, isOutput=False)
    idx2p = nc.declare_dram_parameter("idx2p", [P, IW2], i16, isOutput=False)
    disq1p = nc.declare_dram_parameter("disq1p", [P, NT * D], f32, isOutput=False)
    disq2p = nc.declare_dram_parameter("disq2p", [P, NT * D], f32, isOutput=False)
    disrp = nc.declare_dram_parameter("disrp", [2, NPAIR * P], f32, isOutput=False)
    identp = nc.declare_dram_parameter("identp", [P, P], f32, isOutput=False)
    identrp = nc.declare_dram_parameter("identrp", [P, P], f32r, isOutput=False)
    wb1p = nc.declare_dram_parameter("wb1p", [P, P], f32, isOutput=False)
    wb2p = nc.declare_dram_parameter("wb2p", [P, P], f32, isOutput=False)
    bp1p = nc.declare_dram_parameter("bp1p", [2, P], f32, isOutput=False)
    bp2p = nc.declare_dram_parameter("bp2p", [2, P], f32, isOutput=False)
    out_sh = nc.declare_dram_parameter("out_sh", [NSHP, D], f32, isOutput=True)

    vsh = nc.dram_tensor("vsh", [NSHP, D], f32r)
    vfa = nc.dram_tensor("vfa", [NTA, D], f32r, addr_space="Shared")
    vfb = nc.dram_tensor("vfb", [NTB, D], f32r, addr_space="Shared")
    vtabs = [vfa, vfb]
    rg = [list(range(C))]

    with tile.TileContext(nc) as tc:
        with tc.tile_pool(name="const", bufs=1) as cp, \
             tc.tile_pool(name="gp", bufs=2) as gpool, \
             tc.tile_pool(name="ep", bufs=3) as ep, \
             tc.tile_pool(name="psA", bufs=2, space="PSUM") as psA, \
             tc.tile_pool(name="psB", bufs=2, space="PSUM") as psB, \
             tc.tile_pool(name="psC", bufs=2, space="PSUM") as psC:

            nc.gpsimd.load_library(mlp)

            ident = cp.tile([P, P], f32)
            nc.sync.dma_start(out=ident[:], in_=identp[:, :])
            identr = cp.tile([P, P], f32r)
            nc.sync.dma_start(out=identr[:], in_=identrp[:, :])
            wb1 = cp.tile([P, P], f32)
            nc.sync.dma_start(out=wb1[:], in_=wb1p[:, :])
            wb2 = cp.tile([P, P], f32)
            nc.sync.dma_start(out=wb2[:], in_=wb2p[:, :])
            bt1 = cp.tile([2, P], f32)
            nc.sync.dma_start(out=bt1[:], in_=bp1p[:, :])
            bt2 = cp.tile([2, P], f32)
            nc.sync.dma_start(out=bt2[:], in_=bp2p[:, :])
            dq1 = cp.tile([P, NT * D], f32)
            nc.sync.dma_start(out=dq1[:], in_=disq1p[:, :])
            dq2 = cp.tile([P, NT * D], f32)
            nc.sync.dma_start(out=dq2[:], in_=disq2p[:, :])
            drow = cp.tile([2, NPAIR * P], f32)
            nc.sync.dma_start(out=drow[:], in_=disrp[:, :])
            ones2 = cp.tile([2, P], f32)
            nc.vector.memset(ones2[:], 1.0)
            zrow = cp.tile([1, D], f32)
            nc.vector.memset(zrow[:], 0.0)
            it1 = cp.tile([P, IW1], i16)
            nch = 4
            for ci in range(nch):
                a, b = (IW1 * ci) // nch, (IW1 * (ci + 1)) // nch
                nc.sync.dma_start(out=it1[:, a:b], in_=idx1p[:, a:b])
            it2 = cp.tile([P, IW2], i16)
            for ci in range(nch):
                a, b = (IW2 * ci) // nch, (IW2 * (ci + 1)) // nch
                nc.sync.dma_start(out=it2[:, a:b], in_=idx2p[:, a:b])

            # zero rows of the v tables
            for vt_, zp_ in ((vfa, zposA), (vfb, zposB)):
                for z in zp_:
                    nc.sync.dma_start(out=vt_[int(z):int(z) + 1, :],
                                      in_=zrow[:].bitcast(f32r))

            def layer(L, sched, offs, itab, wins, dq, wblk, bpair, brhs,
                      selfsrc, dest, post_group, vdt):
                for gi, m in enumerate(sched):
                    t0, ntile = m["t0"], m["ntile"]
                    nt64 = ntile * D
                    nblk = int(m["D"].sum()) * ntile
                    acc = psA.tile([P, 4 * D], f32)
                    gbuf = gpool.tile([P, maxblk, D], f32r, tag="g")
                    MAXG = 1024          # max idxs per gather (desc ring)
                    blk0 = 0
                    for (w, c0, ncols, n_idx) in offs[gi]:
                        tabap, base = wins[w]
                        nbw = n_idx // P
                        for ci in range(0, n_idx, MAXG):
                            n_ = min(MAXG, n_idx - ci)
                            nc.gpsimd.dma_gather(
                                gbuf[:, blk0 + ci // P:
                                     blk0 + (ci + n_) // P, :],
                                tabap,
                                itab[:, c0 + ci // 16:c0 + (ci + n_) // 16],
                                n_, n_, D)
                        blk0 += nbw
                    # self rows (own shard, contiguous, pre-scaled on host)
                    st = ep.tile([P, 4, D], f32r, tag="sl")
                    nc.sync.dma_start(
                        out=st[:, :ntile, :],
                        in_=selfsrc(t0, ntile))
                    nc.tensor.matmul(acc[:, :nt64], lhsT=identr[:],
                                     rhs=st[:, :ntile, :],
                                     start=True, stop=(nblk == 0))
                    for b_ in range(0, nblk, ntile):
                        nc.tensor.matmul(
                            acc[:, :nt64], lhsT=identr[:],
                            rhs=gbuf[:, b_:b_ + ntile, :],
                            start=False, stop=(b_ + ntile >= nblk))
                    csb = ep.tile([P, 4 * D], f32, tag="c")
                    nc.vector.tensor_mul(csb[:, :nt64], acc[:, :nt64],
                                         dq[:, t0 * D:(t0 + ntile) * D])
                    # W-stage per pair of tiles
                    npair = ntile // 2
                    for pi in range(npair):
                        po = pi * P
                        tr1 = psB.tile([P, P], f32)
                        nc.tensor.matmul(tr1[:], lhsT=csb[:, po:po + P],
                                         rhs=ident[:], is_transpose=True)
                        ct = ep.tile([P, P], f32, tag="ct")
                        nc.vector.tensor_copy(out=ct[:], in_=tr1[:])
                        pv = psC.tile([P, P], f32)
                        nc.tensor.matmul(pv[:], lhsT=wblk[:], rhs=ct[:],
                                         start=True, stop=False)
                        t_ = t0 + 2 * pi
                        nc.tensor.matmul(pv[:], lhsT=bpair[:], rhs=brhs(t_),
                                         start=False, stop=True)
                        vt_ = ep.tile([P, P], f32, tag="vt")
                        nc.vector.tensor_copy(out=vt_[:], in_=pv[:])
                        tr2 = psB.tile([P, P], f32)
                        nc.tensor.matmul(tr2[:], lhsT=vt_[:], rhs=ident[:],
                                         is_transpose=True)
                        vsb = ep.tile([P, P], vdt, tag="vs%d" % L)
                        nc.vector.tensor_copy(out=vsb[:], in_=tr2[:])
                        d2 = dest(t_, 2)
                        nc.sync.dma_start(out=d2[0:P, :], in_=vsb[:, :D])
                        nc.scalar.dma_start(out=d2[P:2 * P, :],
                                            in_=vsb[:, D:])
                    if ntile % 2:
                        t_ = t0 + 2 * npair
                        po = 2 * npair * D
                        tr1 = psB.tile([P, P], f32)
                        nc.tensor.matmul(tr1[:D, :], lhsT=csb[:, po:po + D],
                                         rhs=ident[:], is_transpose=True)
                        ct = ep.tile([P, P], f32, tag="ct")
                        nc.vector.tensor_copy(out=ct[:D, :], in_=tr1[:D, :])
                        pv = psC.tile([P, P], f32)
                        nc.tensor.matmul(pv[:D, :], lhsT=wblk[:D, :D],
                                         rhs=ct[:D, :], start=True, stop=False)
                        nc.tensor.matmul(pv[:D, :], lhsT=bpair[0:1, :D],
                                         rhs=brhs(t_)[0:1, :],
                                         start=False, stop=True)
                        vt_ = ep.tile([P, P], f32, tag="vt")
                        nc.vector.tensor_copy(out=vt_[:D, :], in_=pv[:D, :])
                        tr2 = psB.tile([P, P], f32)
                        nc.tensor.matmul(tr2[:, :D], lhsT=vt_[:D, :],
                                         rhs=ident[:D, :D], is_transpose=True)
                        vsb = ep.tile([P, P], vdt, tag="vs%d" % L)
                        nc.vector.tensor_copy(out=vsb[:, :D], in_=tr2[:, :D])
                        nc.sync.dma_start(out=dest(t_, 1), in_=vsb[:, :D])
                    if post_group is not None:
                        post_group(gi)

            wins1 = [(t1p[bases1[0]:bases1[0] + WSZ, :], bases1[0]),
                     (t1p[bases1[1]:bases1[1] + WSZ, :], bases1[1]),
                     (t1p[bases1[2]:bases1[2] + WSZ, :], bases1[2]),
                     (t1rp[bases1[3]:bases1[3] + WSZ, :], bases1[3]),
                     (t1rp[bases1[4]:bases1[4] + WSZ, :], bases1[4]),
                     (t1rp[bases1[5]:bases1[5] + WSZ, :], bases1[5])]
            wins2 = [(vfa[bases2[0]:bases2[0] + WSZ, :], bases2[0]),
                     (vfa[bases2[1]:bases2[1] + WSZ, :], bases2[1]),
                     (vfa[bases2[2]:bases2[2] + WSZ, :], bases2[2]),
                     (vfb[bases2[3]:bases2[3] + WSZ, :], bases2[3]),
                     (vfb[bases2[4]:bases2[4] + WSZ, :], bases2[4]),
                     (vfb[bases2[5]:bases2[5] + WSZ, :], bases2[5])]

            TBs = [TB_A, TB_B]
            prs = [prA, prB]
            vbases = [baseA, baseB]

            def fire(gi):
                if gi not in FIRE:
                    return
                for (ci, pi) in FIRE[gi]:
                    TB = TBs[ci]
                    lo, hi = TB[pi] * P, TB[pi + 1] * P
                    ob = int(vbases[ci][pi])
                    nc.gpsimd.collective_compute(
                        "AllGather", mybir.AluOpType.bypass,
                        replica_groups=rg,
                        ins=[vsh[lo:hi, :]],
                        outs=[vtabs[ci][ob:ob + C * (hi - lo), :]])

            layer(1, sched1, offs1, it1, wins1, dq1, wb1, bt1,
                  lambda t: drow[:, (t // 2) * P:(t // 2 + 1) * P],
                  lambda t0, ntile: self1p[t0:t0 + ntile, :, :].transpose([1, 0, 2]),
                  lambda t, ntile: vsh[t * P:(t + ntile) * P, :],
                  fire, f32r)
            layer(2, sched2, offs2, it2, wins2, dq2, wb2, bt2,
                  lambda t: ones2[:, :],
                  lambda t0, ntile: vsh[t0 * P:(t0 + ntile) * P, :]
                                    .rearrange("(q p) f -> q p f", p=P)
                                    .transpose([1, 0, 2]),
                  lambda t, ntile: out_sh[t * P:(t + ntile) * P, :],
                  None, f32)

    nc.compile()

    in_maps = []
    for k in range(C):
        in_maps.append({
            "t1p": T1, "t1rp": T1R,
            "self1p": self1_arr[k].reshape(NT, P, D),
            "idx1p": idx1_host[k], "idx2p": idx2_host[k],
            "disq1p": disq1_arr[k], "disq2p": disq2_arr[k],
            "disrp": disrow_arr[k],
            "identp": np.eye(P, dtype=np.float32),
            "identrp": np.eye(P, dtype=np.float32),
            "wb1p": wblk1, "wb2p": wblk2, "bp1p": bp1, "bp2p": bp2,
        })
    global _compiled
    _compiled = (nc, in_maps)
    res = run_bass_kernel_spmd(nc, in_maps, list(range(C)))
    out = np.empty((N, D), dtype=np.float32)
    for k in range(C):
        out[k * NSH + perms[k]] = res.results[k]["out_sh"][:NSH]
    return out


def profile_last():
    """Re-run the last compiled program with NTFF tracing; returns exec ns."""
    from concourse.bass_utils import run_bass_kernel_spmd
    assert _compiled is not None
    nc, in_maps = _compiled
    r = run_bass_kernel_spmd(nc, in_maps, list(range(C)), trace=True)
    return r.exec_time_ns


# revision 12
# speedup vs baseline: 2.3249x; 2.3249x over previous
"""Two-layer GCN block (PyG GCNConv x2) on 8 trn2 NeuronCores.

Math: out1 = D^-1/2 (A+I) D^-1/2 (x W1) + b1 ; out2 = same on out1 with W2, b2.
Factorization on device (dest-sharded, 6250 dests/core, degree-sorted tiles):
    u    = dis (.) x                        (host; dis = deg^-1/2)
    A[d] = sum_{e: s->d} u[s] + dis_d x_d   (batched dma_gather + PSUM matmuls)
    v    = dis^2 (.) A @ W1 + dis (.) b1    (= dis (.) out1, gathered by layer 2)
    out2 = dis (.) (A2 @ W2) + b2
Gathers use the SWDGE dma_gather instruction (one instruction per
(group, window) instead of one indirect DMA per slot).  int16 gather
indices limit each gather to a 32768-row window, so each layer's table
exists in two copies (layer 1: host-built rotation; layer 2: two
AllGather layouts with different piece boundaries) and six windows,
with per-group window slot counts from a small LP.
"""
import sys
import numpy as np

sys.path.insert(0, '/root/.axon_site')
sys.path.insert(0, '/opt/trn_rl_repo')

N = 50000
E = 800000
D = 64
C = 8
NSH = 6250
P = 128
NT = 49                 # dest tiles per core
NSHP = NT * P           # 6272
WSZ = 32768
GROUPS = [(g * 4, 4) for g in range(12)] + [(48, 1)]
NW = 6                  # gather windows per layer

# layer-1 table: nodes + 5 interspersed zero rows, plus a rotated copy
NZ1 = 5
NTAB1 = N + NZ1         # 50005
ZP1 = np.array([0, 12501, 25002, 37503, 50004], dtype=np.int64)
ROT = 25003
B1 = NTAB1 - WSZ        # 17237

# layer-2 tables: two AllGather layouts (piece-major, rank-major regions
# with a zero row before each region and one at the end)
TB_A = [0, 20, 36, 48, 49]
TB_B = [0, 12, 24, 36, 48, 49]
# fire copy-piece after these group indices (group g covers tiles [4g,4g+4))
FIRE = {2: [(1, 0)], 4: [(0, 0)], 5: [(1, 1)], 8: [(0, 1), (1, 2)],
        11: [(0, 2), (1, 3)], 12: [(0, 3), (1, 4)]}

_compiled = None


def _vlayout(TB):
    """zero positions, region bases, per-core piece rows for a v copy."""
    pr = [(TB[i + 1] - TB[i]) * P for i in range(len(TB) - 1)]
    zpos, base = [], []
    cur = 0
    for i in range(len(pr)):
        zpos.append(cur)
        cur += 1
        base.append(cur)
        cur += C * pr[i]
    zpos.append(cur)
    cur += 1
    return pr, np.array(zpos), np.array(base), cur  # cur = NTAB


def _win_zero(zpos_sorted, b):
    """a zero-row local index inside window [b, b+WSZ)"""
    for z in zpos_sorted:
        if b <= z < b + WSZ:
            return int(z - b)
    raise AssertionError(f"no zero row in window base {b}")


def _lp_slots(maxcnt, nw):
    """min sum D_w s.t. sum_{w in S} D_w >= maxcnt[S] for all subsets S."""
    from scipy.optimize import linprog
    Aub, bub = [], []
    for S in range(1, 1 << nw):
        if maxcnt[S] == 0:
            continue
        Aub.append([-(1.0 if (S >> w) & 1 else 0.0) for w in range(nw)])
        bub.append(-float(maxcnt[S]))
    if not Aub:
        return np.zeros(nw, dtype=np.int64)
    res = linprog(c=np.ones(nw), A_ub=Aub, b_ub=bub,
                  bounds=[(0, None)] * nw, method='highs')
    Ds = np.ceil(res.x - 1e-9).astype(np.int64)
    for _ in range(30):
        ok = True
        for S in range(1, 1 << nw):
            need = maxcnt[S] - sum(Ds[w] for w in range(nw) if (S >> w) & 1)
            if need > 0:
                wmax = max((w for w in range(nw) if (S >> w) & 1),
                           key=lambda w: Ds[w])
                Ds[wmax] += need
                ok = False
        if ok:
            break
    return Ds


_POP = np.array([bin(m).count('1') for m in range(1 << NW)])


def _schedule_layer(posW, bases, rs_by_core, starts_by_core, perms):
    """Per-group window slot counts + per (group, core) source->slot values.

    posW: [NW, N] position of each node in each window's copy coordinates
          (already base-subtracted; -1 if outside the window)
    Returns groups_meta: list of dicts with D (per-window slots) and
    vals[k][w] = int16 [D_w, ntile, P] arrays (index values, pad = zero row).
    """
    inwin = posW >= 0                      # [NW, N]
    masks_all = np.zeros(N, dtype=np.int64)
    for w in range(NW):
        masks_all |= inwin[w].astype(np.int64) << w
    assert (masks_all > 0).all()

    out = []
    for gi, (t0, ntile) in enumerate(GROUPS):
        lo, hi = t0 * P, min((t0 + ntile) * P, NSH)
        # subset maxima over all cores' dests
        cnt = np.zeros((C * (hi - lo), 1 << NW), dtype=np.int64)
        src_lists = []   # per core: (dsts, per-dest source arrays)
        for k in range(C):
            dsts = perms[k][lo:hi]
            st = starts_by_core[k]
            rs = rs_by_core[k]
            row_i = []
            for di, dv in enumerate(dsts):
                srcs = rs[st[dv]:st[dv + 1]]
                row_i.append(srcs)
                if len(srcs):
                    m, c_ = np.unique(masks_all[srcs], return_counts=True)
                    cnt[k * (hi - lo) + di, m] = c_
            src_lists.append((dsts, row_i))
        # zeta transform: cnt[:, S] = #sources with mask subset of S
        for w in range(NW):
            bit = 1 << w
            idx = np.arange(1 << NW)
            sel = (idx & bit) != 0
            cnt[:, idx[sel]] += cnt[:, idx[sel] ^ bit]
        maxcnt = cnt.max(axis=0)
        Ds = _lp_slots(maxcnt, NW)

        # greedy per-dest assignment with caps Ds (bump on failure)
        for _attempt in range(50):
            vals = [[np.full((int(Ds[w]), ntile, P), -1, dtype=np.int64)
                     for w in range(NW)] for _ in range(C)]
            failed = None
            for k in range(C):
                dsts, row_i = src_lists[k]
                for di, srcs in enumerate(row_i):
                    if len(srcs) == 0:
                        continue
                    q, p = di // P, di % P
                    mks = masks_all[srcs]
                    order = np.argsort(_POP[mks], kind='stable')
                    used = np.zeros(NW, dtype=np.int64)
                    for si in order:
                        m = mks[si]
                        best, bestrem = -1, 0
                        for w in range(NW):
                            if (m >> w) & 1:
                                rem = Ds[w] - used[w]
                                if rem > bestrem:
                                    best, bestrem = w, rem
                        if best < 0:
                            failed = m
                            break
                        vals[k][best][used[best], q, p] = posW[best][srcs[si]]
                        used[best] += 1
                    if failed is not None:
                        break
                if failed is not None:
                    break
            if failed is None:
                break
            w_b = max((w for w in range(NW) if (failed >> w) & 1),
                      key=lambda w: Ds[w])
            Ds[w_b] += 1
        assert failed is None, "assignment failed after bumps"
        out.append({"t0": t0, "ntile": ntile, "D": Ds, "vals": vals})
    return out


def _build_idx_host(sched, zlocs):
    """Pack per-(group, window) values into the int16 SBUF index layout."""
    cols_total = sum(8 * int(m["D"][w]) * m["ntile"]
                     for m in sched for w in range(NW))
    idx = np.zeros((C, P, cols_total), dtype=np.int16)
    offs = []     # per group: list of (w, col_off, ncols, num_idxs)
    c0 = 0
    for m in sched:
        ntile = m["ntile"]
        go = []
        for w in range(NW):
            Dw = int(m["D"][w])
            if Dw == 0:
                continue
            ncols = 8 * Dw * ntile
            n_idx = P * Dw * ntile
            for k in range(C):
                v = m["vals"][k][w]          # [Dw, ntile, P]
                v = np.where(v < 0, zlocs[w], v)
                flat = v.reshape(-1).astype(np.int16)      # (s, q, p) order
                blk = flat.reshape(-1, 16).T               # [16, ncols]
                idx[k, :, c0:c0 + ncols] = np.tile(blk, (8, 1))
            go.append((w, c0, ncols, n_idx))
            c0 += ncols
        offs.append(go)
    return idx, offs, cols_total


def kernel(x, edge_index, W1, b1, W2, b2):
    import concourse.bass as bass
    import concourse.bacc as bacc
    import concourse.mybir as mybir
    from concourse import tile
    from concourse.library_config import mlp
    from concourse.bass_utils import run_bass_kernel_spmd

    x = np.asarray(x, dtype=np.float32)
    edge_index = np.asarray(edge_index)
    W1 = np.asarray(W1, dtype=np.float32)
    W2 = np.asarray(W2, dtype=np.float32)
    b1 = np.asarray(b1, dtype=np.float32).reshape(1, D)
    b2 = np.asarray(b2, dtype=np.float32).reshape(1, D)

    row = edge_index[0].astype(np.int64)
    col = edge_index[1].astype(np.int64)
    deg = np.bincount(col, minlength=N).astype(np.float32) + 1.0
    dis = (1.0 / np.sqrt(deg)).astype(np.float32)

    # ---- per-core edge lists (dest-sharded), degree-sorted dest tiles ----
    order = np.argsort(col, kind='stable')
    col_s, row_s = col[order], row[order]
    bounds = np.searchsorted(col_s, np.arange(0, N + 1, NSH))
    perms, pinvs, starts_by_core, rs_by_core = [], [], [], []
    for k in range(C):
        sl = slice(bounds[k], bounds[k + 1])
        lc = col_s[sl] - k * NSH
        dd = np.bincount(lc, minlength=NSH)
        perm = np.argsort(-dd, kind='stable')
        pinv = np.empty(NSH, dtype=np.int64)
        pinv[perm] = np.arange(NSH)
        perms.append(perm)
        pinvs.append(pinv)
        starts_by_core.append(np.searchsorted(lc, np.arange(NSH + 1)))
        rs_by_core.append(row_s[sl])

    # ---- layer-1 tables (u = dis * x, two copies, interspersed zeros) ----
    pos1 = np.delete(np.arange(NTAB1), ZP1)          # node i -> T1 row
    pos1r = (pos1 - ROT) % NTAB1                     # node i -> T1R row
    u = dis[:, None] * x
    T1 = np.zeros((NTAB1, D), dtype=np.float32)
    T1[pos1] = u
    T1R = np.zeros((NTAB1, D), dtype=np.float32)
    T1R[pos1r] = u
    zp1r_sorted = np.sort((ZP1 - ROT) % NTAB1)
    bases1 = [0, B1 // 2, B1, 0, B1 // 2, B1]        # windows 0-2: T1, 3-5: T1R
    zlocs1 = [_win_zero(ZP1, bases1[w]) if w < 3 else
              _win_zero(zp1r_sorted, bases1[w]) for w in range(NW)]
    posW1 = np.full((NW, N), -1, dtype=np.int64)
    for w in range(NW):
        p_ = (pos1 if w < 3 else pos1r) - bases1[w]
        posW1[w] = np.where((p_ >= 0) & (p_ < WSZ), p_, -1)

    # ---- layer-2 tables: two v layouts ----
    prA, zposA, baseA, NTA = _vlayout(TB_A)
    prB, zposB, baseB, NTB = _vlayout(TB_B)
    B2a, B2b = NTA - WSZ, NTB - WSZ

    def vpos(TB, pr, base_arr):
        # node -> position in this copy
        pos = np.empty(N, dtype=np.int64)
        tbp = np.array(TB[1:]) * P
        for k in range(C):
            p = pinvs[k][np.arange(NSH)]
            pc = np.searchsorted(tbp, p, side='right')
            pos[k * NSH:(k + 1) * NSH] = (base_arr[pc] + k * np.array(pr)[pc]
                                          + (p - np.array(TB)[pc] * P))
        return pos

    pos2a = vpos(TB_A, prA, baseA)
    pos2b = vpos(TB_B, prB, baseB)
    bases2 = [0, B2a // 2, B2a, 0, B2b // 2, B2b]
    zlocs2 = [_win_zero(zposA, bases2[w]) if w < 3 else
              _win_zero(zposB, bases2[w]) for w in range(NW)]
    posW2 = np.full((NW, N), -1, dtype=np.int64)
    for w in range(NW):
        p_ = (pos2a if w < 3 else pos2b) - bases2[w]
        posW2[w] = np.where((p_ >= 0) & (p_ < WSZ), p_, -1)

    # ---- schedules + index arrays ----
    sched1 = _schedule_layer(posW1, bases1, rs_by_core, starts_by_core, perms)
    sched2 = _schedule_layer(posW2, bases2, rs_by_core, starts_by_core, perms)
    idx1_host, offs1, IW1 = _build_idx_host(sched1, zlocs1)
    idx2_host, offs2, IW2 = _build_idx_host(sched2, zlocs2)
    maxblk = max(int(m["D"].sum()) * m["ntile"] for m in sched1 + sched2)

    # ---- per-core dense arrays ----
    NPAIR = (NT + 1) // 2                            # 25 pair slots
    self1_arr = np.zeros((C, NSHP, D), dtype=np.float32)
    disq1_arr = np.zeros((C, P, NT * D), dtype=np.float32)
    disq2_arr = np.zeros((C, P, NT * D), dtype=np.float32)
    disrow_arr = np.zeros((C, 2, NPAIR * P), dtype=np.float32)
    for k in range(C):
        dval = dis[k * NSH + perms[k]]               # dis by sorted position
        self1_arr[k, :NSH] = dval[:, None] * x[k * NSH + perms[k]]
        dpad = np.zeros(NSHP, dtype=np.float32)
        dpad[:NSH] = dval
        dt_ = dpad.reshape(NT, P)                    # [tile, dest-in-tile]
        disq1_arr[k] = np.repeat(dt_.T ** 2, D, axis=1)
        disq2_arr[k] = np.repeat(dt_.T, D, axis=1)
        disrow_arr[k, 0, :] = dt_[0::2].reshape(-1)
        disrow_arr[k, 1, :NT // 2 * P] = dt_[1::2].reshape(-1)

    wblk1 = np.zeros((P, P), dtype=np.float32)
    wblk1[:D, :D] = W1
    wblk1[D:, D:] = W1
    wblk2 = np.zeros((P, P), dtype=np.float32)
    wblk2[:D, :D] = W2
    wblk2[D:, D:] = W2
    bp1 = np.zeros((2, P), dtype=np.float32)
    bp1[0, :D] = b1[0]
    bp1[1, D:] = b1[0]
    bp2 = np.zeros((2, P), dtype=np.float32)
    bp2[0, :D] = b2[0]
    bp2[1, D:] = b2[0]

    # ---- device program ----
    nc = bacc.Bacc(None, target_bir_lowering=False, num_swdge_queues=4)
    dt = mybir.dt
    f32, f32r, i16 = dt.float32, dt.float32r, dt.int16
    t1p = nc.declare_dram_parameter("t1p", [NTAB1, D], f32r, isOutput=False)
    t1rp = nc.declare_dram_parameter("t1rp", [NTAB1, D], f32r, isOutput=False)
    self1p = nc.declare_dram_parameter("self1p", [NT, P, D], f32r, isOutput=False)
    idx1p = nc.declare_dram_parameter("idx1p", [P, IW1], i16, isOutput=False)
    idx2p = nc.declare_dram_parameter("idx2p", [P, IW2], i16, isOutput=False)
    disq1p = nc.declare_dram_parameter("disq1p", [P, NT * D], f32, isOutput=False)
    disq2p = nc.declare_dram_parameter("disq2p", [P, NT * D], f32, isOutput=False)
    disrp = nc.declare_dram_parameter("disrp", [2, NPAIR * P], f32, isOutput=False)
    identp = nc.declare_dram_parameter("identp", [P, P], f32, isOutput=False)
    identrp = nc.declare_dram_parameter("identrp", [P, P], f32r, isOutput=False)
    wb1p = nc.declare_dram_parameter("wb1p", [P, P], f32, isOutput=False)
    wb2p = nc.declare_dram_parameter("wb2p", [P, P], f32, isOutput=False)
    bp1p = nc.declare_dram_parameter("bp1p", [2, P], f32, isOutput=False)
    bp2p = nc.declare_dram_parameter("bp2p", [2, P], f32, isOutput=False)
    out_sh = nc.declare_dram_parameter("out_sh", [NSHP, D], f32, isOutput=True)

    vsh = nc.dram_tensor("vsh", [NSHP, D], f32r)
    vfa = nc.dram_tensor("vfa", [NTA, D], f32r, addr_space="Shared")
    vfb = nc.dram_tensor("vfb", [NTB, D], f32r, addr_space="Shared")
    vtabs = [vfa, vfb]
    rg = [list(range(C))]

    with tile.TileContext(nc) as tc:
        with tc.tile_pool(name="const", bufs=1) as cp, \
             tc.tile_pool(name="gp", bufs=2) as gpool, \
             tc.tile_pool(name="ep", bufs=3) as ep, \
             tc.tile_pool(name="psA", bufs=2, space="PSUM") as psA, \
             tc.tile_pool(name="psB", bufs=2, space="PSUM") as psB, \
             tc.tile_pool(name="psC", bufs=2, space="PSUM") as psC:

            nc.gpsimd.load_library(mlp)

            ident = cp.tile([P, P], f32)
            nc.sync.dma_start(out=ident[:], in_=identp[:, :])
            identr = cp.tile([P, P], f32r)
            nc.sync.dma_start(out=identr[:], in_=identrp[:, :])
            wb1 = cp.tile([P, P], f32)
            nc.sync.dma_start(out=wb1[:], in_=wb1p[:, :])
            wb2 = cp.tile([P, P], f32)
            nc.sync.dma_start(out=wb2[:], in_=wb2p[:, :])
            bt1 = cp.tile([2, P], f32)
            nc.sync.dma_start(out=bt1[:], in_=bp1p[:, :])
            bt2 = cp.tile([2, P], f32)
            nc.sync.dma_start(out=bt2[:], in_=bp2p[:, :])
            dq1 = cp.tile([P, NT * D], f32)
            nc.sync.dma_start(out=dq1[:], in_=disq1p[:, :])
            dq2 = cp.tile([P, NT * D], f32)
            nc.sync.dma_start(out=dq2[:], in_=disq2p[:, :])
            drow = cp.tile([2, NPAIR * P], f32)
            nc.sync.dma_start(out=drow[:], in_=disrp[:, :])
            ones2 = cp.tile([2, P], f32)
            nc.vector.memset(ones2[:], 1.0)
            zrow = cp.tile([1, D], f32)
            nc.vector.memset(zrow[:], 0.0)
            it1 = cp.tile([P, IW1], i16)
            nch = 4
            for ci in range(nch):
                a, b = (IW1 * ci) // nch, (IW1 * (ci + 1)) // nch
                nc.sync.dma_start(out=it1[:, a:b], in_=idx1p[:, a:b])
            it2 = cp.tile([P, IW2], i16)
            for ci in range(nch):
                a, b = (IW2 * ci) // nch, (IW2 * (ci + 1)) // nch
                nc.sync.dma_start(out=it2[:, a:b], in_=idx2p[:, a:b])

            # zero rows of the v tables
            for vt_, zp_ in ((vfa, zposA), (vfb, zposB)):
                for z in zp_:
                    nc.sync.dma_start(out=vt_[int(z):int(z) + 1, :],
                                      in_=zrow[:].bitcast(f32r))

            def layer(L, sched, offs, itab, wins, dq, wblk, bpair, brhs,
                      selfsrc, dest, post_group, vdt):
                for gi, m in enumerate(sched):
                    t0, ntile = m["t0"], m["ntile"]
                    nt64 = ntile * D
                    nblk = int(m["D"].sum()) * ntile
                    acc = psA.tile([P, 4 * D], f32)
                    gbuf = gpool.tile([P, maxblk, D], f32r, tag="g")
                    MAXG = 1024          # max idxs per gather (desc ring)
                    blk0 = 0
                    for (w, c0, ncols, n_idx) in offs[gi]:
                        tabap, base = wins[w]
                        nbw = n_idx // P
                        for ci in range(0, n_idx, MAXG):
                            n_ = min(MAXG, n_idx - ci)
                            nc.gpsimd.dma_gather(
                                gbuf[:, blk0 + ci // P:
                                     blk0 + (ci + n_) // P, :],
                                tabap,
                                itab[:, c0 + ci // 16:c0 + (ci + n_) // 16],
                                n_, n_, D)
                        blk0 += nbw
                    # self rows (own shard, contiguous, pre-scaled on host)
                    st = ep.tile([P, 4, D], f32r, tag="sl")
                    nc.sync.dma_start(
                        out=st[:, :ntile, :],
                        in_=selfsrc(t0, ntile))
                    nc.tensor.matmul(acc[:, :nt64], lhsT=identr[:],
                                     rhs=st[:, :ntile, :],
                                     start=True, stop=(nblk == 0))
                    for b_ in range(0, nblk, ntile):
                        nc.tensor.matmul(
                            acc[:, :nt64], lhsT=identr[:],
                            rhs=gbuf[:, b_:b_ + ntile, :],
                            start=False, stop=(b_ + ntile >= nblk))
                    csb = ep.tile([P, 4 * D], f32, tag="c")
                    nc.vector.tensor_mul(csb[:, :nt64], acc[:, :nt64],
                                         dq[:, t0 * D:(t0 + ntile) * D])
                    # W-stage per pair of tiles
                    npair = ntile // 2
                    for pi in range(npair):
                        po = pi * P
                        tr1 = psB.tile([P, P], f32)
                        nc.tensor.matmul(tr1[:], lhsT=csb[:, po:po + P],
                                         rhs=ident[:], is_transpose=True)
                        ct = ep.tile([P, P], f32, tag="ct")
                        nc.vector.tensor_copy(out=ct[:], in_=tr1[:])
                        pv = psC.tile([P, P], f32)
                        nc.tensor.matmul(pv[:], lhsT=wblk[:], rhs=ct[:],
                                         start=True, stop=False)
                        t_ = t0 + 2 * pi
                        nc.tensor.matmul(pv[:], lhsT=bpair[:], rhs=brhs(t_),
                                         start=False, stop=True)
                        vt_ = ep.tile([P, P], f32, tag="vt")
                        nc.vector.tensor_copy(out=vt_[:], in_=pv[:])
                        tr2 = psB.tile([P, P], f32)
                        nc.tensor.matmul(tr2[:], lhsT=vt_[:], rhs=ident[:],
                                         is_transpose=True)
                        vsb = ep.tile([P, P], vdt, tag="vs%d" % L)
                        nc.vector.tensor_copy(out=vsb[:], in_=tr2[:])
                        d2 = dest(t_, 2)
                        nc.sync.dma_start(out=d2[0:P, :], in_=vsb[:, :D])
                        nc.scalar.dma_start(out=d2[P:2 * P, :],
                                            in_=vsb[:, D:])
                    if ntile % 2:
                        t_ = t0 + 2 * npair
                        po = 2 * npair * D
                        tr1 = psB.tile([P, P], f32)
                        nc.tensor.matmul(tr1[:D, :], lhsT=csb[:, po:po + D],
                                         rhs=ident[:], is_transpose=True)
                        ct = ep.tile([P, P], f32, tag="ct")
                        nc.vector.tensor_copy(out=ct[:D, :], in_=tr1[:D, :])
                        pv = psC.tile([P, P], f32)
                        nc.tensor.matmul(pv[:D, :], lhsT=wblk[:D, :D],
                                         rhs=ct[:D, :], start=True, stop=False)
                        nc.tensor.matmul(pv[:D, :], lhsT=bpair[0:1, :D],
                                         rhs=brhs(t_)[0:1, :],
                                         start=False, stop=True)
                        vt_ = ep.tile([P, P], f32, tag="vt")
                        nc.vector.tensor_copy(out=vt_[:D, :], in_=pv[:D, :])
                        tr2 = psB.tile([P, P], f32)
                        nc.tensor.matmul(tr2[:, :D], lhsT=vt_[:D, :],
                                         rhs=ident[:D, :D], is_transpose=True)
                        vsb = ep.tile([P, P], vdt, tag="vs%d" % L)
                        nc.vector.tensor_copy(out=vsb[:, :D], in_=tr2[:, :D])
                        nc.sync.dma_start(out=dest(t_, 1), in_=vsb[:, :D])
                    if post_group is not None:
                        post_group(gi)

            wins1 = [(t1p[bases1[0]:bases1[0] + WSZ, :], bases1[0]),
                     (t1p[bases1[1]:bases1[1] + WSZ, :], bases1[1]),
                     (t1p[bases1[2]:bases1[2] + WSZ, :], bases1[2]),
                     (t1rp[bases1[3]:bases1[3] + WSZ, :], bases1[3]),
                     (t1rp[bases1[4]:bases1[4] + WSZ, :], bases1[4]),
                     (t1rp[bases1[5]:bases1[5] + WSZ, :], bases1[5])]
            wins2 = [(vfa[bases2[0]:bases2[0] + WSZ, :], bases2[0]),
                     (vfa[bases2[1]:bases2[1] + WSZ, :], bases2[1]),
                     (vfa[bases2[2]:bases2[2] + WSZ, :], bases2[2]),
                     (vfb[bases2[3]:bases2[3] + WSZ, :], bases2[3]),
                     (vfb[bases2[4]:bases2[4] + WSZ, :], bases2[4]),
                     (vfb[bases2[5]:bases2[5] + WSZ, :], bases2[5])]

            TBs = [TB_A, TB_B]
            prs = [prA, prB]
            vbases = [baseA, baseB]

            def fire(gi):
                if gi not in FIRE:
                    return
                for (ci, pi) in FIRE[gi]:
                    TB = TBs[ci]
                    lo, hi = TB[pi] * P, TB[pi + 1] * P
                    ob = int(vbases[ci][pi])
                    nc.gpsimd.collective_compute(
                        "AllGather", mybir.AluOpType.bypass,
                        replica_groups=rg,
                        ins=[vsh[lo:hi, :]],
                        outs=[vtabs[ci][ob:ob + C * (hi - lo), :]])

            layer(1, sched1, offs1, it1, wins1, dq1, wb1, bt1,
                  lambda t: drow[:, (t // 2) * P:(t // 2 + 1) * P],
                  lambda t0, ntile: self1p[t0:t0 + ntile, :, :].transpose([1, 0, 2]),
                  lambda t, ntile: vsh[t * P:(t + ntile) * P, :],
                  fire, f32r)
            layer(2, sched2, offs2, it2, wins2, dq2, wb2, bt2,
                  lambda t: ones2[:, :],
                  lambda t0, ntile: vsh[t0 * P:(t0 + ntile) * P, :]
                                    .rearrange("(q p) f -> q p f", p=P)
                                    .transpose([1, 0, 2]),
                  lambda t, ntile: out_sh[t * P:(t + ntile) * P, :],
                  None, f32)

    # Spread gathers across the 4 SWDGE queues, consistent with the
    # tile scheduler's DMASW semaphore-lane rotation (lane i -> queue i%4)
    # so each DMASW sem only ever sees one queue.
    from concourse.tile_sem_assignment import PROC_NAME_TO_IDX
    lane_of = {PROC_NAME_TO_IDX[f"DMASW{i}"]: i for i in range(8)}
    for blk in nc.main_func.blocks:
        for inst in blk.instructions:
            if isinstance(inst, mybir.InstDMAGatherAnt):
                lane = lane_of.get(inst.bass_scheduled_proc)
                if lane is not None:
                    inst.queue_num = lane % 4

    nc.compile()

    in_maps = []
    for k in range(C):
        in_maps.append({
            "t1p": T1, "t1rp": T1R,
            "self1p": self1_arr[k].reshape(NT, P, D),
            "idx1p": idx1_host[k], "idx2p": idx2_host[k],
            "disq1p": disq1_arr[k], "disq2p": disq2_arr[k],
            "disrp": disrow_arr[k],
            "identp": np.eye(P, dtype=np.float32),
            "identrp": np.eye(P, dtype=np.float32),
            "wb1p": wblk1, "wb2p": wblk2, "bp1p": bp1, "bp2p": bp2,
        })
    global _compiled
    _compiled = (nc, in_maps)
    res = run_bass_kernel_spmd(nc, in_maps, list(range(C)))
    out = np.empty((N, D), dtype=np.float32)
    for k in range(C):
        out[k * NSH + perms[k]] = res.results[k]["out_sh"][:NSH]
    return out


def profile_last():
    """Re-run the last compiled program with NTFF tracing; returns exec ns."""
    from concourse.bass_utils import run_bass_kernel_spmd
    assert _compiled is not None
    nc, in_maps = _compiled
    r = run_bass_kernel_spmd(nc, in_maps, list(range(C)), trace=True)
    return r.exec_time_ns


# revision 15
# speedup vs baseline: 2.8709x; 1.2349x over previous
"""Two-layer GCN block (PyG GCNConv x2) on 8 trn2 NeuronCores.

Math: out1 = D^-1/2 (A+I) D^-1/2 (x W1) + b1 ; out2 = same on out1 with W2, b2.
Factorization on device (dest-sharded, 6250 dests/core, degree-sorted tiles):
    u    = dis (.) x                        (host; dis = deg^-1/2)
    A[d] = sum_{e: s->d} u[s] + dis_d x_d   (batched dma_gather + PSUM matmuls)
    v    = dis^2 (.) A @ W1 + dis (.) b1    (= dis (.) out1, gathered by layer 2)
    out2 = dis (.) (A2 @ W2) + b2
Gathers use the SWDGE dma_gather instruction (one instruction per
(group, window) instead of one indirect DMA per slot).  int16 gather
indices limit each gather to a 32768-row window, so each layer's table
exists in two copies (layer 1: host-built rotation; layer 2: two
AllGather layouts with different piece boundaries) and six windows,
with per-group window slot counts from a small LP.
"""
import sys
import numpy as np

sys.path.insert(0, '/root/.axon_site')
sys.path.insert(0, '/opt/trn_rl_repo')

N = 50000
E = 800000
D = 64
C = 8
NSH = 6250
P = 128
NT = 49                 # dest tiles per core
NSHP = NT * P           # 6272
WSZ = 32768
GROUPS = [(g * 4, 4) for g in range(12)] + [(48, 1)]
NW = 6                  # gather windows per layer

# layer-1 table: nodes + 5 interspersed zero rows, plus a rotated copy
NZ1 = 5
NTAB1 = N + NZ1         # 50005
ZP1 = np.array([0, 12501, 25002, 37503, 50004], dtype=np.int64)
ROT = 25003
B1 = NTAB1 - WSZ        # 17237

# layer-2 tables: two AllGather layouts (piece-major, rank-major regions
# with a zero row before each region and one at the end)
TB_A = [0, 20, 36, 48, 49]
TB_B = [0, 12, 24, 36, 48, 49]
# fire copy-piece after these group indices (group g covers tiles [4g,4g+4))
FIRE = {2: [(1, 0)], 4: [(0, 0)], 5: [(1, 1)], 8: [(0, 1), (1, 2)],
        11: [(0, 2), (1, 3)], 12: [(0, 3), (1, 4)]}

_compiled = None


def _vlayout(TB):
    """zero positions, region bases, per-core piece rows for a v copy."""
    pr = [(TB[i + 1] - TB[i]) * P for i in range(len(TB) - 1)]
    zpos, base = [], []
    cur = 0
    for i in range(len(pr)):
        zpos.append(cur)
        cur += 1
        base.append(cur)
        cur += C * pr[i]
    zpos.append(cur)
    cur += 1
    return pr, np.array(zpos), np.array(base), cur  # cur = NTAB


def _win_zeros(zpos_sorted, b):
    """sorted window-relative zero-row positions inside [b, b+WSZ)"""
    za = np.asarray(zpos_sorted)
    z = za[(za >= b) & (za < b + WSZ)] - b
    assert len(z) > 0, f"no zero row in window base {b}"
    return np.sort(z)


def _lp_slots(maxcnt, nw):
    """min sum D_w s.t. sum_{w in S} D_w >= maxcnt[S] for all subsets S."""
    from scipy.optimize import linprog
    Aub, bub = [], []
    for S in range(1, 1 << nw):
        if maxcnt[S] == 0:
            continue
        Aub.append([-(1.0 if (S >> w) & 1 else 0.0) for w in range(nw)])
        bub.append(-float(maxcnt[S]))
    if not Aub:
        return np.zeros(nw, dtype=np.int64)
    res = linprog(c=np.ones(nw), A_ub=Aub, b_ub=bub,
                  bounds=[(0, None)] * nw, method='highs')
    Ds = np.ceil(res.x - 1e-9).astype(np.int64)
    for _ in range(30):
        ok = True
        for S in range(1, 1 << nw):
            need = maxcnt[S] - sum(Ds[w] for w in range(nw) if (S >> w) & 1)
            if need > 0:
                wmax = max((w for w in range(nw) if (S >> w) & 1),
                           key=lambda w: Ds[w])
                Ds[wmax] += need
                ok = False
        if ok:
            break
    return Ds


_POP = np.array([bin(m).count('1') for m in range(1 << NW)])


def _schedule_layer(posW, bases, rs_by_core, starts_by_core, perms):
    """Per-group window slot counts + per (group, core) source->slot values.

    posW: [NW, N] position of each node in each window's copy coordinates
          (already base-subtracted; -1 if outside the window)
    Returns groups_meta: list of dicts with D (per-window slots) and
    vals[k][w] = int16 [D_w, ntile, P] arrays (index values, pad = zero row).
    """
    inwin = posW >= 0                      # [NW, N]
    masks_all = np.zeros(N, dtype=np.int64)
    for w in range(NW):
        masks_all |= inwin[w].astype(np.int64) << w
    assert (masks_all > 0).all()

    out = []
    for gi, (t0, ntile) in enumerate(GROUPS):
        lo, hi = t0 * P, min((t0 + ntile) * P, NSH)
        # subset maxima over all cores' dests
        cnt = np.zeros((C * (hi - lo), 1 << NW), dtype=np.int64)
        src_lists = []   # per core: (dsts, per-dest source arrays)
        for k in range(C):
            dsts = perms[k][lo:hi]
            st = starts_by_core[k]
            rs = rs_by_core[k]
            row_i = []
            for di, dv in enumerate(dsts):
                srcs = rs[st[dv]:st[dv + 1]]
                row_i.append(srcs)
                if len(srcs):
                    m, c_ = np.unique(masks_all[srcs], return_counts=True)
                    cnt[k * (hi - lo) + di, m] = c_
            src_lists.append((dsts, row_i))
        # zeta transform: cnt[:, S] = #sources with mask subset of S
        for w in range(NW):
            bit = 1 << w
            idx = np.arange(1 << NW)
            sel = (idx & bit) != 0
            cnt[:, idx[sel]] += cnt[:, idx[sel] ^ bit]
        maxcnt = cnt.max(axis=0)
        Ds = _lp_slots(maxcnt, NW)

        # per-dest b-matching with caps Ds (Hall-feasible by construction;
        # augmenting step handles greedy dead-ends, bump only as backstop)
        for _attempt in range(50):
            vals = [[np.full((int(Ds[w]), ntile, P), -1, dtype=np.int64)
                     for w in range(NW)] for _ in range(C)]
            failed = None
            for k in range(C):
                dsts, row_i = src_lists[k]
                for di, srcs in enumerate(row_i):
                    if len(srcs) == 0:
                        continue
                    q, p = di // P, di % P
                    mks = masks_all[srcs]
                    order = np.argsort(_POP[mks], kind='stable')
                    used = np.zeros(NW, dtype=np.int64)
                    assign = {}          # si -> w

                    def try_place(w, seen):
                        if used[w] < Ds[w]:
                            used[w] += 1
                            return True
                        for sj, wj in list(assign.items()):
                            if wj != w:
                                continue
                            mj = mks[sj]
                            for w2 in range(NW):
                                if w2 != w and ((mj >> w2) & 1) \
                                        and w2 not in seen:
                                    if try_place(w2, seen | {w2}):
                                        assign[sj] = w2
                                        return True
                        return False

                    for si in order:
                        m = mks[si]
                        cands = [w for w in range(NW) if (m >> w) & 1]
                        best = max(cands, key=lambda w: Ds[w] - used[w])
                        if used[best] < Ds[best]:
                            used[best] += 1
                            assign[si] = best
                            continue
                        ok = False
                        for w in cands:
                            if try_place(w, {w}):
                                assign[si] = w
                                ok = True
                                break
                        if not ok:
                            failed = m
                            break
                    if failed is not None:
                        break
                    slots = np.zeros(NW, dtype=np.int64)
                    for si, w in assign.items():
                        vals[k][w][slots[w], q, p] = posW[w][srcs[si]]
                        slots[w] += 1
                    # sort each window's slot values by position (locality
                    # + narrow per-chunk in_ap ranges)
                    for w in range(NW):
                        nsl = int(slots[w])
                        if nsl > 1:
                            col = vals[k][w][:nsl, q, p]
                            vals[k][w][:nsl, q, p] = np.sort(col)
                if failed is not None:
                    break
            if failed is None:
                break
            w_b = max((w for w in range(NW) if (failed >> w) & 1),
                      key=lambda w: Ds[w])
            Ds[w_b] += 1
        assert failed is None, "assignment failed after bumps"
        out.append({"t0": t0, "ntile": ntile, "D": Ds, "vals": vals})
    return out


MAXG = 1024             # max idxs per dma_gather (SWDGE desc ring capacity)


def _build_idx_host(sched, zposw):
    """Pack values into the int16 SBUF index layout, chunked at MAXG idxs.

    zposw[w] = sorted window-relative positions of zero rows in window w.
    Each chunk's in_ap is narrowed to [row_lo, row_lo+nrows) of the window
    (indices rebased), so deps and DMA locality are tight.
    Returns idx[C,128,IW], offs (per group: list of chunk dicts), IW.
    """
    cols_total = sum(8 * int(m["D"][w]) * m["ntile"]
                     for m in sched for w in range(NW))
    idx = np.zeros((C, P, cols_total), dtype=np.int16)
    offs = []
    c0 = 0
    for m in sched:
        ntile = m["ntile"]
        go = []
        blk0 = 0
        for w in range(NW):
            Dw = int(m["D"][w])
            if Dw == 0:
                continue
            n_tot = P * Dw * ntile
            vws = [m["vals"][k][w].reshape(-1) for k in range(C)]  # (s,q,p)
            for ci in range(0, n_tot, MAXG):
                n_ = min(MAXG, n_tot - ci)
                sl = slice(ci, ci + n_)
                seg = np.stack([v[sl] for v in vws])       # [C, n_]
                real = seg >= 0
                if real.any():
                    mn, mx = int(seg[real].min()), int(seg[real].max())
                else:
                    mn = mx = int(zposw[w][0])
                if not real.all():
                    zi = np.searchsorted(zposw[w], mn)
                    zc = []
                    if zi < len(zposw[w]):
                        zc.append(int(zposw[w][zi]))
                    if zi > 0:
                        zc.append(int(zposw[w][zi - 1]))
                    assert zc, "no zero row available"
                    z = min(zc, key=lambda v: max(0, mn - v) + max(0, v - mx))
                    mn, mx = min(mn, z), max(mx, z)
                    seg = np.where(real, seg, z)
                seg = seg - mn
                ncols = n_ // 16
                for k in range(C):
                    blk = seg[k].astype(np.int16).reshape(-1, 16).T
                    idx[k, :, c0:c0 + ncols] = np.tile(blk, (8, 1))
                go.append({"w": w, "c0": c0, "ncols": ncols, "n": n_,
                           "lo": mn, "nrows": mx - mn + 1,
                           "b0": blk0 + ci // P})
                c0 += ncols
            blk0 += Dw * ntile
        offs.append(go)
    return idx, offs, cols_total


def kernel(x, edge_index, W1, b1, W2, b2):
    import concourse.bass as bass
    import concourse.bacc as bacc
    import concourse.mybir as mybir
    from concourse import tile
    from concourse.library_config import mlp
    from concourse.bass_utils import run_bass_kernel_spmd

    x = np.asarray(x, dtype=np.float32)
    edge_index = np.asarray(edge_index)
    W1 = np.asarray(W1, dtype=np.float32)
    W2 = np.asarray(W2, dtype=np.float32)
    b1 = np.asarray(b1, dtype=np.float32).reshape(1, D)
    b2 = np.asarray(b2, dtype=np.float32).reshape(1, D)

    row = edge_index[0].astype(np.int64)
    col = edge_index[1].astype(np.int64)
    deg = np.bincount(col, minlength=N).astype(np.float32) + 1.0
    dis = (1.0 / np.sqrt(deg)).astype(np.float32)

    # ---- per-core edge lists (dest-sharded), degree-sorted dest tiles ----
    order = np.argsort(col, kind='stable')
    col_s, row_s = col[order], row[order]
    bounds = np.searchsorted(col_s, np.arange(0, N + 1, NSH))
    perms, pinvs, starts_by_core, rs_by_core = [], [], [], []
    for k in range(C):
        sl = slice(bounds[k], bounds[k + 1])
        lc = col_s[sl] - k * NSH
        dd = np.bincount(lc, minlength=NSH)
        perm = np.argsort(-dd, kind='stable')
        pinv = np.empty(NSH, dtype=np.int64)
        pinv[perm] = np.arange(NSH)
        perms.append(perm)
        pinvs.append(pinv)
        starts_by_core.append(np.searchsorted(lc, np.arange(NSH + 1)))
        rs_by_core.append(row_s[sl])

    # ---- layer-1 tables (u = dis * x, two copies, interspersed zeros) ----
    pos1 = np.delete(np.arange(NTAB1), ZP1)          # node i -> T1 row
    pos1r = (pos1 - ROT) % NTAB1                     # node i -> T1R row
    u = dis[:, None] * x
    T1 = np.zeros((NTAB1, D), dtype=np.float32)
    T1[pos1] = u
    T1R = np.zeros((NTAB1, D), dtype=np.float32)
    T1R[pos1r] = u
    zp1r_sorted = np.sort((ZP1 - ROT) % NTAB1)
    bases1 = [0, B1 // 2, B1, 0, B1 // 2, B1]        # windows 0-2: T1, 3-5: T1R
    zposw1 = [_win_zeros(ZP1, bases1[w]) if w < 3 else
              _win_zeros(zp1r_sorted, bases1[w]) for w in range(NW)]
    posW1 = np.full((NW, N), -1, dtype=np.int64)
    for w in range(NW):
        p_ = (pos1 if w < 3 else pos1r) - bases1[w]
        posW1[w] = np.where((p_ >= 0) & (p_ < WSZ), p_, -1)

    # ---- layer-2 tables: two v layouts ----
    prA, zposA, baseA, NTA = _vlayout(TB_A)
    prB, zposB, baseB, NTB = _vlayout(TB_B)
    B2a, B2b = NTA - WSZ, NTB - WSZ

    def vpos(TB, pr, base_arr):
        # node -> position in this copy
        pos = np.empty(N, dtype=np.int64)
        tbp = np.array(TB[1:]) * P
        for k in range(C):
            p = pinvs[k][np.arange(NSH)]
            pc = np.searchsorted(tbp, p, side='right')
            pos[k * NSH:(k + 1) * NSH] = (base_arr[pc] + k * np.array(pr)[pc]
                                          + (p - np.array(TB)[pc] * P))
        return pos

    pos2a = vpos(TB_A, prA, baseA)
    pos2b = vpos(TB_B, prB, baseB)
    bases2 = [0, B2a // 2, B2a, 0, B2b // 2, B2b]
    zposw2 = [_win_zeros(zposA, bases2[w]) if w < 3 else
              _win_zeros(zposB, bases2[w]) for w in range(NW)]
    posW2 = np.full((NW, N), -1, dtype=np.int64)
    for w in range(NW):
        p_ = (pos2a if w < 3 else pos2b) - bases2[w]
        posW2[w] = np.where((p_ >= 0) & (p_ < WSZ), p_, -1)

    # ---- schedules + index arrays ----
    sched1 = _schedule_layer(posW1, bases1, rs_by_core, starts_by_core, perms)
    sched2 = _schedule_layer(posW2, bases2, rs_by_core, starts_by_core, perms)
    idx1_host, offs1, IW1 = _build_idx_host(sched1, zposw1)
    idx2_host, offs2, IW2 = _build_idx_host(sched2, zposw2)
    maxblk = max(int(m["D"].sum()) * m["ntile"] for m in sched1 + sched2)

    # ---- per-core dense arrays ----
    NPAIR = (NT + 1) // 2                            # 25 pair slots
    self1_arr = np.zeros((C, NSHP, D), dtype=np.float32)
    disq1_arr = np.zeros((C, P, NT * D), dtype=np.float32)
    disq2_arr = np.zeros((C, P, NT * D), dtype=np.float32)
    disrow_arr = np.zeros((C, 2, NPAIR * P), dtype=np.float32)
    for k in range(C):
        dval = dis[k * NSH + perms[k]]               # dis by sorted position
        self1_arr[k, :NSH] = dval[:, None] * x[k * NSH + perms[k]]
        dpad = np.zeros(NSHP, dtype=np.float32)
        dpad[:NSH] = dval
        dt_ = dpad.reshape(NT, P)                    # [tile, dest-in-tile]
        disq1_arr[k] = np.repeat(dt_.T ** 2, D, axis=1)
        disq2_arr[k] = np.repeat(dt_.T, D, axis=1)
        disrow_arr[k, 0, :] = dt_[0::2].reshape(-1)
        disrow_arr[k, 1, :NT // 2 * P] = dt_[1::2].reshape(-1)

    wblk1 = np.zeros((P, P), dtype=np.float32)
    wblk1[:D, :D] = W1
    wblk1[D:, D:] = W1
    wblk2 = np.zeros((P, P), dtype=np.float32)
    wblk2[:D, :D] = W2
    wblk2[D:, D:] = W2
    bp1 = np.zeros((2, P), dtype=np.float32)
    bp1[0, :D] = b1[0]
    bp1[1, D:] = b1[0]
    bp2 = np.zeros((2, P), dtype=np.float32)
    bp2[0, :D] = b2[0]
    bp2[1, D:] = b2[0]

    # ---- device program ----
    nc = bacc.Bacc(None, target_bir_lowering=False, num_swdge_queues=4)
    dt = mybir.dt
    f32, f32r, i16 = dt.float32, dt.float32r, dt.int16
    t1p = nc.declare_dram_parameter("t1p", [NTAB1, D], f32r, isOutput=False)
    t1rp = nc.declare_dram_parameter("t1rp", [NTAB1, D], f32r, isOutput=False)
    self1p = nc.declare_dram_parameter("self1p", [NT, P, D], f32r, isOutput=False)
    idx1p = nc.declare_dram_parameter("idx1p", [P, IW1], i16, isOutput=False)
    idx2p = nc.declare_dram_parameter("idx2p", [P, IW2], i16, isOutput=False)
    disq1p = nc.declare_dram_parameter("disq1p", [P, NT * D], f32, isOutput=False)
    disq2p = nc.declare_dram_parameter("disq2p", [P, NT * D], f32, isOutput=False)
    disrp = nc.declare_dram_parameter("disrp", [2, NPAIR * P], f32, isOutput=False)
    identp = nc.declare_dram_parameter("identp", [P, P], f32, isOutput=False)
    identrp = nc.declare_dram_parameter("identrp", [P, P], f32r, isOutput=False)
    wb1p = nc.declare_dram_parameter("wb1p", [P, P], f32, isOutput=False)
    wb2p = nc.declare_dram_parameter("wb2p", [P, P], f32, isOutput=False)
    bp1p = nc.declare_dram_parameter("bp1p", [2, P], f32, isOutput=False)
    bp2p = nc.declare_dram_parameter("bp2p", [2, P], f32, isOutput=False)
    out_sh = nc.declare_dram_parameter("out_sh", [NSHP, D], f32, isOutput=True)

    vsh = nc.dram_tensor("vsh", [NSHP, D], f32r)
    vfa = nc.dram_tensor("vfa", [NTA, D], f32r, addr_space="Shared")
    vfb = nc.dram_tensor("vfb", [NTB, D], f32r, addr_space="Shared")
    vtabs = [vfa, vfb]
    rg = [list(range(C))]

    with tile.TileContext(nc) as tc:
        with tc.tile_pool(name="const", bufs=1) as cp, \
             tc.tile_pool(name="gp", bufs=2) as gpool, \
             tc.tile_pool(name="ep", bufs=3) as ep, \
             tc.tile_pool(name="psA", bufs=2, space="PSUM") as psA, \
             tc.tile_pool(name="psB", bufs=2, space="PSUM") as psB, \
             tc.tile_pool(name="psC", bufs=2, space="PSUM") as psC:

            nc.gpsimd.load_library(mlp)

            ident = cp.tile([P, P], f32)
            nc.sync.dma_start(out=ident[:], in_=identp[:, :])
            identr = cp.tile([P, P], f32r)
            nc.sync.dma_start(out=identr[:], in_=identrp[:, :])
            wb1 = cp.tile([P, P], f32)
            nc.sync.dma_start(out=wb1[:], in_=wb1p[:, :])
            wb2 = cp.tile([P, P], f32)
            nc.sync.dma_start(out=wb2[:], in_=wb2p[:, :])
            bt1 = cp.tile([2, P], f32)
            nc.sync.dma_start(out=bt1[:], in_=bp1p[:, :])
            bt2 = cp.tile([2, P], f32)
            nc.sync.dma_start(out=bt2[:], in_=bp2p[:, :])
            dq1 = cp.tile([P, NT * D], f32)
            nc.sync.dma_start(out=dq1[:], in_=disq1p[:, :])
            dq2 = cp.tile([P, NT * D], f32)
            nc.sync.dma_start(out=dq2[:], in_=disq2p[:, :])
            drow = cp.tile([2, NPAIR * P], f32)
            nc.sync.dma_start(out=drow[:], in_=disrp[:, :])
            ones2 = cp.tile([2, P], f32)
            nc.vector.memset(ones2[:], 1.0)
            zrow = cp.tile([1, D], f32)
            nc.vector.memset(zrow[:], 0.0)
            it1 = cp.tile([P, IW1], i16)
            nch = 4
            for ci in range(nch):
                a, b = (IW1 * ci) // nch, (IW1 * (ci + 1)) // nch
                nc.sync.dma_start(out=it1[:, a:b], in_=idx1p[:, a:b])
            it2 = cp.tile([P, IW2], i16)
            for ci in range(nch):
                a, b = (IW2 * ci) // nch, (IW2 * (ci + 1)) // nch
                nc.sync.dma_start(out=it2[:, a:b], in_=idx2p[:, a:b])

            # zero rows of the v tables
            for vt_, zp_ in ((vfa, zposA), (vfb, zposB)):
                for z in zp_:
                    nc.sync.dma_start(out=vt_[int(z):int(z) + 1, :],
                                      in_=zrow[:].bitcast(f32r))

            def layer(L, sched, offs, itab, wins, dq, wblk, bpair, brhs,
                      selfsrc, dest, post_group, vdt):
                for gi, m in enumerate(sched):
                    t0, ntile = m["t0"], m["ntile"]
                    nt64 = ntile * D
                    nblk = int(m["D"].sum()) * ntile
                    acc = psA.tile([P, 4 * D], f32)
                    gbuf = gpool.tile([P, maxblk, D], f32r, tag="g")
                    for ch in offs[gi]:
                        tab, base = wins[ch["w"]]
                        lo = base + ch["lo"]
                        nc.gpsimd.dma_gather(
                            gbuf[:, ch["b0"]:ch["b0"] + ch["n"] // P, :],
                            tab[lo:lo + ch["nrows"], :],
                            itab[:, ch["c0"]:ch["c0"] + ch["ncols"]],
                            ch["n"], ch["n"], D)
                    # self rows (own shard, contiguous, pre-scaled on host)
                    st = ep.tile([P, 4, D], f32r, tag="sl")
                    nc.sync.dma_start(
                        out=st[:, :ntile, :],
                        in_=selfsrc(t0, ntile))
                    nc.tensor.matmul(acc[:, :nt64], lhsT=identr[:],
                                     rhs=st[:, :ntile, :],
                                     start=True, stop=(nblk == 0))
                    for b_ in range(0, nblk, ntile):
                        nc.tensor.matmul(
                            acc[:, :nt64], lhsT=identr[:],
                            rhs=gbuf[:, b_:b_ + ntile, :],
                            start=False, stop=(b_ + ntile >= nblk))
                    csb = ep.tile([P, 4 * D], f32, tag="c")
                    nc.vector.tensor_mul(csb[:, :nt64], acc[:, :nt64],
                                         dq[:, t0 * D:(t0 + ntile) * D])
                    # W-stage per pair of tiles
                    npair = ntile // 2
                    for pi in range(npair):
                        po = pi * P
                        tr1 = psB.tile([P, P], f32)
                        nc.tensor.matmul(tr1[:], lhsT=csb[:, po:po + P],
                                         rhs=ident[:], is_transpose=True)
                        ct = ep.tile([P, P], f32, tag="ct")
                        nc.vector.tensor_copy(out=ct[:], in_=tr1[:])
                        pv = psC.tile([P, P], f32)
                        nc.tensor.matmul(pv[:], lhsT=wblk[:], rhs=ct[:],
                                         start=True, stop=False)
                        t_ = t0 + 2 * pi
                        nc.tensor.matmul(pv[:], lhsT=bpair[:], rhs=brhs(t_),
                                         start=False, stop=True)
                        vt_ = ep.tile([P, P], f32, tag="vt")
                        nc.vector.tensor_copy(out=vt_[:], in_=pv[:])
                        tr2 = psB.tile([P, P], f32)
                        nc.tensor.matmul(tr2[:], lhsT=vt_[:], rhs=ident[:],
                                         is_transpose=True)
                        vsb = ep.tile([P, P], vdt, tag="vs%d" % L)
                        nc.vector.tensor_copy(out=vsb[:], in_=tr2[:])
                        d2 = dest(t_, 2)
                        nc.sync.dma_start(out=d2[0:P, :], in_=vsb[:, :D])
                        nc.scalar.dma_start(out=d2[P:2 * P, :],
                                            in_=vsb[:, D:])
                    if ntile % 2:
                        t_ = t0 + 2 * npair
                        po = 2 * npair * D
                        tr1 = psB.tile([P, P], f32)
                        nc.tensor.matmul(tr1[:D, :], lhsT=csb[:, po:po + D],
                                         rhs=ident[:], is_transpose=True)
                        ct = ep.tile([P, P], f32, tag="ct")
                        nc.vector.tensor_copy(out=ct[:D, :], in_=tr1[:D, :])
                        pv = psC.tile([P, P], f32)
                        nc.tensor.matmul(pv[:D, :], lhsT=wblk[:D, :D],
                                         rhs=ct[:D, :], start=True, stop=False)
                        nc.tensor.matmul(pv[:D, :], lhsT=bpair[0:1, :D],
                                         rhs=brhs(t_)[0:1, :],
                                         start=False, stop=True)
                        vt_ = ep.tile([P, P], f32, tag="vt")
                        nc.vector.tensor_copy(out=vt_[:D, :], in_=pv[:D, :])
                        tr2 = psB.tile([P, P], f32)
                        nc.tensor.matmul(tr2[:, :D], lhsT=vt_[:D, :],
                                         rhs=ident[:D, :D], is_transpose=True)
                        vsb = ep.tile([P, P], vdt, tag="vs%d" % L)
                        nc.vector.tensor_copy(out=vsb[:, :D], in_=tr2[:, :D])
                        nc.sync.dma_start(out=dest(t_, 1), in_=vsb[:, :D])
                    if post_group is not None:
                        post_group(gi)

            wins1 = [(t1p, bases1[w]) if w < 3 else (t1rp, bases1[w])
                     for w in range(NW)]
            wins2 = [(vfa, bases2[w]) if w < 3 else (vfb, bases2[w])
                     for w in range(NW)]

            TBs = [TB_A, TB_B]
            prs = [prA, prB]
            vbases = [baseA, baseB]

            def fire(gi):
                if gi not in FIRE:
                    return
                for (ci, pi) in FIRE[gi]:
                    TB = TBs[ci]
                    lo, hi = TB[pi] * P, TB[pi + 1] * P
                    ob = int(vbases[ci][pi])
                    nc.gpsimd.collective_compute(
                        "AllGather", mybir.AluOpType.bypass,
                        replica_groups=rg,
                        ins=[vsh[lo:hi, :]],
                        outs=[vtabs[ci][ob:ob + C * (hi - lo), :]])

            layer(1, sched1, offs1, it1, wins1, dq1, wb1, bt1,
                  lambda t: drow[:, (t // 2) * P:(t // 2 + 1) * P],
                  lambda t0, ntile: self1p[t0:t0 + ntile, :, :].transpose([1, 0, 2]),
                  lambda t, ntile: vsh[t * P:(t + ntile) * P, :],
                  fire, f32r)
            layer(2, sched2, offs2, it2, wins2, dq2, wb2, bt2,
                  lambda t: ones2[:, :],
                  lambda t0, ntile: vsh[t0 * P:(t0 + ntile) * P, :]
                                    .rearrange("(q p) f -> q p f", p=P)
                                    .transpose([1, 0, 2]),
                  lambda t, ntile: out_sh[t * P:(t + ntile) * P, :],
                  None, f32)

    # Spread gathers across the 4 SWDGE queues, consistent with the
    # tile scheduler's DMASW semaphore-lane rotation (lane i -> queue i%4)
    # so each DMASW sem only ever sees one queue.
    from concourse.tile_sem_assignment import PROC_NAME_TO_IDX
    lane_of = {PROC_NAME_TO_IDX[f"DMASW{i}"]: i for i in range(8)}
    for blk in nc.main_func.blocks:
        for inst in blk.instructions:
            if isinstance(inst, mybir.InstDMAGatherAnt):
                lane = lane_of.get(inst.bass_scheduled_proc)
                if lane is not None:
                    inst.queue_num = lane % 4

    nc.compile()

    in_maps = []
    for k in range(C):
        in_maps.append({
            "t1p": T1, "t1rp": T1R,
            "self1p": self1_arr[k].reshape(NT, P, D),
            "idx1p": idx1_host[k], "idx2p": idx2_host[k],
            "disq1p": disq1_arr[k], "disq2p": disq2_arr[k],
            "disrp": disrow_arr[k],
            "identp": np.eye(P, dtype=np.float32),
            "identrp": np.eye(P, dtype=np.float32),
            "wb1p": wblk1, "wb2p": wblk2, "bp1p": bp1, "bp2p": bp2,
        })
    global _compiled
    _compiled = (nc, in_maps)
    res = run_bass_kernel_spmd(nc, in_maps, list(range(C)))
    out = np.empty((N, D), dtype=np.float32)
    for k in range(C):
        out[k * NSH + perms[k]] = res.results[k]["out_sh"][:NSH]
    return out


def profile_last():
    """Re-run the last compiled program with NTFF tracing; returns exec ns."""
    from concourse.bass_utils import run_bass_kernel_spmd
    assert _compiled is not None
    nc, in_maps = _compiled
    r = run_bass_kernel_spmd(nc, in_maps, list(range(C)), trace=True)
    return r.exec_time_ns
